# revision 13
# baseline (speedup 1.0000x reference)
"""Cross-attention without softmax on 8 trn2 NeuronCores.

Reference computes out = (X Wq^T) (C Wk^T)^T (C Wv^T) * D^-0.5 per batch.
With no softmax the product reassociates:

    out_b = X_b @ P2_b,  P2_b = U G_b W2,  U = Wq^T Wk,
    G_b = C_b^T C_b,     W2 = D^-0.5 Wv^T

U and W2 are weight-only and precomputed on the host. The device computes
G (32 accumulating 128x128 matmuls alternating between two PSUM banks),
then V = G W2 as two accumulating matmuls off the two banks (G is
symmetric so the bank copies are their own lhsT), P2 = U V, and finally
out = X P2 as 16 matmuls whose lhsT slices come straight from a
host-pre-transposed X — no on-device transposes, no Q' intermediate.

The TRN2 PE only reaches its 2.4GHz p-state after ~3us of continuous
execution and drops back on long stalls, so the tensor program front-runs
the first ctx DMA with dummy "warmup" matmuls and keeps short filler
bursts inside the V/P2 chain gaps; all real matmuls then issue at
~56ns/tile instead of ~107ns.

Sharding: batch (4) x query-half (2) -> 8 cores; each core redundantly
computes its batch's G (no collectives). I/O is pre-cast to bf16 on the
host; accumulation stays fp32 in PSUM.

DMA: per-queue throughput is ~150GB/s, so the ~1.6MB of per-core input
is balanced across all three issue queues (sync HW, scalar HW, gpsimd
SW), ctx first (it gates the V chain), xt behind it; the four output
stores are spread over the three queues as well. ctx row-tiles use the
permuted grouping (partition p holds rows base + p*r + j) so ctx DMA
runs >=512B-contiguous per partition; G's row-sum is invariant to the
permutation. Out rows are stored in a device-friendly permuted order
(dev row g*512+p*4+j holds true row g*512+j*128+p, 1KB-contiguous
stores) and un-permuted on the host.
"""

import os
import sys
import types

import numpy as np

_TRN_REPO = "/opt/trn_rl_repo"
if _TRN_REPO not in sys.path and not any("trn_rl_repo" in p for p in sys.path):
    sys.path.insert(0, _TRN_REPO)

import ml_dtypes  # noqa: E402

import concourse.bass as bass  # noqa: E402
import concourse.mybir as mybir  # noqa: E402
from concourse import bacc  # noqa: E402
from concourse.bass_utils import run_bass_kernel_spmd  # noqa: E402

B, SQ, SKV, D = 4, 4096, 4096, 128
N_CORES = 8
SQ_SHARD = SQ // (N_CORES // B)  # 2048
SCALE = float(D) ** -0.5
F32 = mybir.dt.float32
BF16 = mybir.dt.bfloat16

_CACHE: dict = {}


def _install_axon_ntff_shim():
    try:
        import antenv.axon_hooks  # noqa: F401

        return
    except Exception:
        pass
    try:
        from trn_agent_boot.trn_boot import _ntff_profile_via_ctypes

        import antenv

        hook = _ntff_profile_via_ctypes("/opt/axon/libaxon_pjrt.so")
        mod = types.ModuleType("antenv.axon_hooks")
        mod._hook = hook
        mod.get_axon_ntff_profile_hook = lambda: mod._hook

        def _set(h):
            mod._hook = h

        mod.set_axon_ntff_profile_hook = _set
        antenv.axon_hooks = mod
        sys.modules["antenv.axon_hooks"] = mod
    except Exception:
        pass

    try:
        import concourse.bass_utils as bu

        bu.upload_artifacts = lambda tmpdir: f"file://{tmpdir}"
    except Exception:
        pass


# ctx chunks (rows/128), DRAM-contiguous ranges assigned round-robin to the
# three DMA queues: sync gets 0,3; scalar 1,4; gpsimd 2,5 (after w).
CTX_R = (6, 6, 6, 5, 5, 4)
NCC = len(CTX_R)
Q_SYNC, Q_SCAL, Q_GPS = (0, 3), (1, 4), (2, 5)
# PE consumes chunks in expected arrival order
PE_CTX_ORDER = (0, 1, 2, 3, 4, 5)

# PE clock-ramp tuning (see fill() in build_raw)
N_WARM = int(os.environ.get("KERNEL_WARMUP", "21"))
N_CHAIN = int(os.environ.get("KERNEL_CHAINFILL", "3"))


def build_raw():
    """Hand-scheduled raw-bass kernel. Per-core inputs:
    xt (128, 2048) = X-shard transposed, ctx (4096, 128),
    w (128, 256) = [U^T | W2]; output out (2048, 128), permuted rows.

    Queues (FIFO per queue; one semaphore per queue, +16 per DMA):
      sync  HW (s_q1): ctx c0, c3, xt half 0      -> 16, 32, 48
      scalar HW (s_q2): ctx c1, c4, xt half 1     -> 16, 32, 48
      gpsimd SW (s_q3): w, ctx c2, c5             -> 16, 32, 48
    Stores: o0 sync, o1+o3 scalar, o2 gpsimd (all inc s_st by 16).

    Cumulative schedules (value after the op):
      PE (s_pe): G 1-32 (bank A even, bank B odd), V 33-34, P2 35,
                 out 36-51 (4 groups of 4, banks b4-b7)
      vector (s_dve): gaA cast 1, v 2, p2 3, o0 4, o2 5
      scalar (s_sc): gaB cast 1 (o1/o3 copies precede their own stores
                 in scalar program order; no semaphore needed)

    PSUM banks: b0/b1 = G even/odd accumulators; b2 = V; b3 = P2;
    b4-b7 = out groups 0-3 (b7 also absorbs warmup/filler matmuls).
    """
    from contextlib import ExitStack

    cdt = BF16

    nc = bacc.Bacc(None, target_bir_lowering=False, debug=False)
    xt_ext = nc.declare_dram_parameter("xt", [D, SQ_SHARD], cdt, isOutput=False)
    c_ext = nc.declare_dram_parameter("ctx", [SKV, D], cdt, isOutput=False)
    w_ext = nc.declare_dram_parameter("w", [D, 2 * D], cdt, isOutput=False)
    out_ext = nc.declare_dram_parameter("out", [SQ_SHARD, D], cdt, isOutput=True)

    NOG = 4  # out store groups
    OT = SQ_SHARD // 128 // NOG  # 4 tiles per group

    ctx_view = []
    row = 0
    for r in CTX_R:
        ctx_view.append(
            c_ext[row : row + 128 * r, :].rearrange("(p r) d -> p r d", p=128)
        )
        row += 128 * r
    out_view = [
        out_ext[g * 512 : (g + 1) * 512, :].rearrange("(p r) d -> p r d", p=128)
        for g in range(NOG)
    ]

    es = ExitStack()
    _n = [0]

    def sb(shape, dt, name=None):
        _n[0] += 1
        return es.enter_context(nc.sbuf_tensor(name or f"sb{_n[0]}", shape, dt))

    def pst(shape, dt, name=None):
        _n[0] += 1
        return es.enter_context(nc.psum_tensor(name or f"ps{_n[0]}", shape, dt))

    def sem(name):
        return es.enter_context(nc.semaphore(name))

    with es:
        w_sb = sb([D, 2 * D], cdt, "w_sb")
        cc = [sb([128, r, D], cdt, f"cc{i}") for i, r in enumerate(CTX_R)]
        xt_sb = sb([D, SQ_SHARD], cdt, "xt_sb")
        warm_sb = sb([D, D], cdt, "warm_sb")
        ga_bf = sb([D, D], cdt, "ga_bf")
        gb_bf = sb([D, D], cdt, "gb_bf")
        v_sb = sb([D, D], cdt, "v_sb")
        p2_sb = sb([D, D], cdt, "p2_sb")
        o_sb = [sb([128, OT, D], cdt, f"o_sb{i}") for i in range(NOG)]

        ga_ps = pst([128, 512], F32)  # b0
        gb_ps = pst([128, 512], F32)  # b1
        v_ps = pst([128, 512], F32)  # b2
        p2_ps = pst([128, 512], F32)  # b3
        o_ps = [pst([128, 512], F32) for _ in range(NOG)]  # b4-b7

        s_pe = sem("s_pe")
        s_dve = sem("s_dve")
        s_sc = sem("s_sc")
        s_q1 = sem("s_q1")
        s_q2 = sem("s_q2")
        s_q3 = sem("s_q3")
        s_st = sem("s_st")

        # chunk index -> (queue sem, threshold)
        chunk_gate = {}
        for qsem, chunks, base in (
            (s_q1, Q_SYNC, 0),
            (s_q2, Q_SCAL, 0),
            (s_q3, Q_GPS, 16),  # w is first on the gpsimd queue
        ):
            for pos, ci in enumerate(chunks):
                chunk_gate[ci] = (qsem, base + 16 * (pos + 1))

        UT = w_sb[:, :D]
        W2 = w_sb[:, D:]

        with nc.Block() as block:

            @block.sync
            def _(sync):
                for i in Q_SYNC:
                    nc.sync.dma_start(cc[i][:], ctx_view[i]).then_inc(s_q1, 16)
                nc.sync.dma_start(
                    xt_sb[:, : SQ_SHARD // 2], xt_ext[:, : SQ_SHARD // 2]
                ).then_inc(s_q1, 16)
                nc.sync.wait_ge(s_dve, 4)  # o0 copied
                nc.sync.dma_start(out_view[0], o_sb[0][:]).then_inc(s_st, 16)
                nc.sync.wait_ge(s_st, 64)

            @block.scalar
            def _(sc):
                for i in Q_SCAL:
                    nc.scalar.dma_start(cc[i][:], ctx_view[i]).then_inc(
                        s_q2, 16
                    )
                nc.scalar.dma_start(
                    xt_sb[:, SQ_SHARD // 2 :], xt_ext[:, SQ_SHARD // 2 :]
                ).then_inc(s_q2, 16)
                nc.scalar.wait_ge(s_pe, 32)  # G bank B done
                nc.scalar.copy(gb_bf[:], gb_ps[:, :128]).then_inc(s_sc, 1)
                nc.scalar.wait_ge(s_pe, 43)  # out group 1 done (b5)
                nc.scalar.copy(
                    o_sb[1][:].rearrange("p n d -> p (n d)"), o_ps[1][:]
                )
                nc.scalar.dma_start(out_view[1], o_sb[1][:]).then_inc(s_st, 16)
                nc.scalar.wait_ge(s_pe, 51)  # out group 3 done (b7)
                nc.scalar.copy(
                    o_sb[3][:].rearrange("p n d -> p (n d)"), o_ps[3][:]
                )
                nc.scalar.dma_start(out_view[3], o_sb[3][:]).then_inc(s_st, 16)
                nc.scalar.wait_ge(s_st, 64)

            @block.gpsimd
            def _(gp):
                nc.gpsimd.dma_start(w_sb[:], w_ext[:]).then_inc(s_q3, 16)
                for i in Q_GPS:
                    nc.gpsimd.dma_start(cc[i][:], ctx_view[i]).then_inc(
                        s_q3, 16
                    )
                nc.gpsimd.wait_ge(s_dve, 5)  # o2 copied
                nc.gpsimd.dma_start(out_view[2], o_sb[2][:]).then_inc(s_st, 16)
                nc.gpsimd.wait_ge(s_st, 64)

            @block.tensor
            def _(te):
                def fill(n):
                    # PE DVFS: full clock only after ~3us of continuous
                    # execution, and long stalls drop it back. Dummy matmuls
                    # on scratch data ramp the clock during DMA waits and
                    # hold it through the V/P2 chain gaps. Results land in
                    # b7, which out group 3 overwrites later.
                    for _ in range(n):
                        nc.tensor.matmul(
                            o_ps[3][:, :128],
                            warm_sb[:],
                            warm_sb[:],
                            start=True,
                            stop=True,
                        )

                fill(N_WARM)
                # G: alternate PSUM banks per matmul so consecutive matmuls
                # pipeline instead of serializing on one bank's accumulator
                m = 0
                n_mm = sum(CTX_R)
                for i in PE_CTX_ORDER:
                    qsem, thr = chunk_gate[i]
                    nc.tensor.wait_ge(qsem, thr)
                    for j in range(CTX_R[i]):
                        bank = ga_ps if m % 2 == 0 else gb_ps
                        nc.tensor.matmul(
                            bank[:, :128],
                            cc[i][:, j, :],
                            cc[i][:, j, :],
                            start=(m < 2),
                            stop=(m >= n_mm - 2),
                        ).then_inc(s_pe, 1)
                        m += 1
                # V = G W2 = (Ga + Gb) W2, accumulated from the two bank
                # copies (G symmetric -> bf16 bank copies are their own lhsT)
                fill(N_CHAIN)  # hold clock while vector/scalar cast banks
                nc.tensor.wait_ge(s_dve, 1)  # ga cast (vector)
                nc.tensor.wait_ge(s_sc, 1)  # gb cast (scalar)
                nc.tensor.matmul(
                    v_ps[:, :128], ga_bf[:], W2, start=True, stop=False
                ).then_inc(s_pe, 1)  # 33
                nc.tensor.matmul(
                    v_ps[:, :128], gb_bf[:], W2, start=False, stop=True
                ).then_inc(s_pe, 1)  # 34
                fill(N_CHAIN)  # hold clock while vector copies V
                # P2 = U V
                nc.tensor.wait_ge(s_dve, 2)  # v copied
                nc.tensor.matmul(
                    p2_ps[:, :128], UT, v_sb[:], start=True, stop=True
                ).then_inc(s_pe, 1)  # 35
                fill(N_CHAIN)  # hold clock while vector copies P2
                # out = X P2: lhsT slices straight from host-transposed X
                nc.tensor.wait_ge(s_dve, 3)  # p2 copied
                nc.tensor.wait_ge(s_q1, 48)  # xt half 0
                for k in range(16):  # 36-51
                    if k == 8:
                        nc.tensor.wait_ge(s_q2, 48)  # xt half 1
                    nc.tensor.matmul(
                        o_ps[k // 4][:, (k % 4) * D : (k % 4 + 1) * D],
                        xt_sb[:, k * 128 : (k + 1) * 128],
                        p2_sb[:],
                        start=True,
                        stop=True,
                    ).then_inc(s_pe, 1)

            @block.vector
            def _(ve):
                nc.vector.wait_ge(s_pe, 31)  # G bank A done (last even m=30)
                nc.vector.tensor_copy(ga_bf[:], ga_ps[:, :128]).then_inc(
                    s_dve, 1
                )
                nc.vector.wait_ge(s_pe, 34)
                nc.vector.tensor_copy(v_sb[:], v_ps[:, :128]).then_inc(
                    s_dve, 1
                )
                nc.vector.wait_ge(s_pe, 35)
                nc.vector.tensor_copy(p2_sb[:], p2_ps[:, :128]).then_inc(
                    s_dve, 1
                )
                nc.vector.wait_ge(s_pe, 39)  # out group 0 done (b4)
                nc.vector.tensor_copy(
                    o_sb[0][:].rearrange("p n d -> p (n d)"), o_ps[0][:]
                ).then_inc(s_dve, 1)
                nc.vector.wait_ge(s_pe, 47)  # out group 2 done (b6)
                nc.vector.tensor_copy(
                    o_sb[2][:].rearrange("p n d -> p (n d)"), o_ps[2][:]
                ).then_inc(s_dve, 1)

    nc.compile()
    return nc


def _get_nc():
    if "nc" not in _CACHE:
        _CACHE["nc"] = build_raw()
    return _CACHE["nc"]


def _prep_in_maps(inputs: dict):
    bf16 = ml_dtypes.bfloat16
    context = np.ascontiguousarray(inputs["context"]).astype(bf16)
    X = np.ascontiguousarray(inputs["X"]).astype(np.float32)
    Wq = np.ascontiguousarray(inputs["Wq"]).astype(np.float32)
    Wk = np.ascontiguousarray(inputs["Wk"]).astype(np.float32)
    Wv = np.ascontiguousarray(inputs["Wv"]).astype(np.float32)

    UT = Wk.T @ Wq  # (Wq^T Wk)^T
    W2 = SCALE * Wv.T
    w_host = np.ascontiguousarray(np.concatenate([UT, W2], axis=1).astype(bf16))

    Xb = X.astype(bf16)
    in_maps = []
    for c in range(N_CORES):
        b, h = divmod(c, 2)
        xt = np.ascontiguousarray(Xb[b, h * SQ_SHARD : (h + 1) * SQ_SHARD, :].T)
        in_maps.append(
            {"xt": xt, "ctx": np.ascontiguousarray(context[b]), "w": w_host}
        )
    return in_maps


def _unpermute(dev: np.ndarray) -> np.ndarray:
    # dev row g*512 + p*4 + j holds true row g*512 + j*128 + p
    return dev.reshape(4, 128, 4, D).transpose(0, 2, 1, 3).reshape(SQ_SHARD, D)


def _run(inputs: dict, trace: bool = False, **kw):
    in_maps = _prep_in_maps(inputs)
    nc = _get_nc()
    res = run_bass_kernel_spmd(
        nc, in_maps, core_ids=list(range(N_CORES)), trace=trace, **kw
    )
    out = np.empty((B, SQ, D), dtype=np.float32)
    for c in range(N_CORES):
        b, h = divmod(c, 2)
        out[b, h * SQ_SHARD : (h + 1) * SQ_SHARD, :] = _unpermute(
            res.results[c]["out"]
        ).astype(np.float32)
    return out, res


def kernel(**inputs: np.ndarray) -> np.ndarray:
    if os.environ.get("BASS_TRACE"):
        _install_axon_ntff_shim()
    try:
        out, _ = _run(inputs, trace=False)
    except Exception:
        # transient NRT device errors have been observed once across many
        # runs; one retry on a fresh execution
        out, _ = _run(inputs, trace=False)
    return out


if __name__ == "__main__":
    rng = np.random.default_rng(0)
    ins = {
        "context": rng.standard_normal((B, SKV, D)).astype(np.float32),
        "X": rng.standard_normal((B, SQ, D)).astype(np.float32),
        "Wq": (rng.standard_normal((D, D)) / np.sqrt(D)).astype(np.float32),
        "Wk": (rng.standard_normal((D, D)) / np.sqrt(D)).astype(np.float32),
        "Wv": (rng.standard_normal((D, D)) / np.sqrt(D)).astype(np.float32),
    }
    got = kernel(**ins)
    q = ins["X"] @ ins["Wq"].T
    k = ins["context"] @ ins["Wk"].T
    v = ins["context"] @ ins["Wv"].T
    w = np.einsum("bse,bte->bst", q, k) * SCALE
    want = np.einsum("bst,bte->bse", w, v)
    rel = np.linalg.norm(got - want) / np.linalg.norm(want)
    print("rel err vs numpy:", rel)


# revision 17
# speedup vs baseline: 1.0280x; 1.0280x over previous
"""Cross-attention without softmax on 8 trn2 NeuronCores.

Reference computes out = (X Wq^T) (C Wk^T)^T (C Wv^T) * D^-0.5 per batch.
With no softmax the product reassociates:

    out_b = X_b @ P2_b,  P2_b = U G_b W2,  U = Wq^T Wk,
    G_b = C_b^T C_b,     W2 = D^-0.5 Wv^T

U and W2 are weight-only and precomputed on the host. The device computes
G (32 accumulating 128x128 matmuls alternating between two PSUM banks),
then V = G W2 as two accumulating matmuls off the two banks (G is
symmetric so the bank copies are their own lhsT), P2 = U V, and finally
out = X P2 as 16 matmuls whose lhsT slices come straight from a
host-pre-transposed X — no on-device transposes, no Q' intermediate.

The TRN2 PE only reaches its 2.4GHz p-state after ~3us of continuous
execution and drops back on long stalls, so the tensor program front-runs
the first ctx DMA with dummy "warmup" matmuls and keeps short filler
bursts inside the V/P2 chain gaps; all real matmuls then issue at
~56ns/tile instead of ~107ns.

Sharding: batch (4) x query-half (2) -> 8 cores; each core redundantly
computes its batch's G (no collectives). I/O is pre-cast to bf16 on the
host; accumulation stays fp32 in PSUM.

DMA: per-queue throughput is ~150GB/s, so the ~1.6MB of per-core input
is balanced across all three issue queues (sync HW, scalar HW, gpsimd
SW), ctx first (it gates the V chain), xt behind it; the four output
stores are spread over the three queues as well. ctx row-tiles use the
permuted grouping (partition p holds rows base + p*r + j) so ctx DMA
runs >=512B-contiguous per partition; G's row-sum is invariant to the
permutation. Out rows are stored in a device-friendly permuted order
(dev row g*512+p*4+j holds true row g*512+j*128+p, 1KB-contiguous
stores) and un-permuted on the host.
"""

import os
import sys
import types

import numpy as np

_TRN_REPO = "/opt/trn_rl_repo"
if _TRN_REPO not in sys.path and not any("trn_rl_repo" in p for p in sys.path):
    sys.path.insert(0, _TRN_REPO)

import ml_dtypes  # noqa: E402

import concourse.bass as bass  # noqa: E402
import concourse.mybir as mybir  # noqa: E402
from concourse import bacc  # noqa: E402
from concourse.bass_utils import run_bass_kernel_spmd  # noqa: E402

B, SQ, SKV, D = 4, 4096, 4096, 128
N_CORES = 8
SQ_SHARD = SQ // (N_CORES // B)  # 2048
SCALE = float(D) ** -0.5
F32 = mybir.dt.float32
BF16 = mybir.dt.bfloat16

_CACHE: dict = {}


def _install_axon_ntff_shim():
    try:
        import antenv.axon_hooks  # noqa: F401

        return
    except Exception:
        pass
    try:
        from trn_agent_boot.trn_boot import _ntff_profile_via_ctypes

        import antenv

        hook = _ntff_profile_via_ctypes("/opt/axon/libaxon_pjrt.so")
        mod = types.ModuleType("antenv.axon_hooks")
        mod._hook = hook
        mod.get_axon_ntff_profile_hook = lambda: mod._hook

        def _set(h):
            mod._hook = h

        mod.set_axon_ntff_profile_hook = _set
        antenv.axon_hooks = mod
        sys.modules["antenv.axon_hooks"] = mod
    except Exception:
        pass

    try:
        import concourse.bass_utils as bu

        bu.upload_artifacts = lambda tmpdir: f"file://{tmpdir}"
    except Exception:
        pass


# ctx chunks (rows/128), DRAM-contiguous ranges. The gpsimd SW queue is only
# ~70-110GB/s, so ctx rides the two ~145GB/s HW queues exclusively:
# sync gets chunks 0,2 then xt half 0; scalar gets 1,3,5,4; gpsimd w + xt1.
CTX_R = (8, 8, 6, 6, 2, 2)
NCC = len(CTX_R)
Q_SYNC, Q_SCAL = (0, 2), (1, 3, 5, 4)
# PE consumes chunks in expected arrival order
PE_CTX_ORDER = (0, 1, 2, 3, 5, 4)

# PE clock-ramp tuning (see fill() in build_raw)
N_WARM = int(os.environ.get("KERNEL_WARMUP", "28"))
N_CHAIN = int(os.environ.get("KERNEL_CHAINFILL", "3"))


def build_raw():
    """Hand-scheduled raw-bass kernel. Per-core inputs:
    xt (128, 2048) = X-shard transposed, ctx (4096, 128),
    w (128, 256) = [U^T | W2]; output out (2048, 128), permuted rows.

    Queues (FIFO per queue; one semaphore per queue, +16 per DMA):
      sync  HW (s_q1): ctx c0, c3, xt half 0      -> 16, 32, 48
      scalar HW (s_q2): ctx c1, c4, xt half 1     -> 16, 32, 48
      gpsimd SW (s_q3): w, ctx c2, c5             -> 16, 32, 48
    Stores: o0 sync, o1+o3 scalar, o2 gpsimd (all inc s_st by 16).

    Cumulative schedules (value after the op):
      PE (s_pe): G 1-32 (bank A even, bank B odd), V 33-34, P2 35,
                 out 36-51 (4 groups of 4, banks b4-b7)
      vector (s_dve): gaA cast 1, v 2, p2 3, o0 4, o2 5
      scalar (s_sc): gaB cast 1 (o1/o3 copies precede their own stores
                 in scalar program order; no semaphore needed)

    PSUM banks: b0/b1 = G even/odd accumulators; b2 = V; b3 = P2;
    b4-b7 = out groups 0-3 (b7 also absorbs warmup/filler matmuls).
    """
    from contextlib import ExitStack

    cdt = BF16

    nc = bacc.Bacc(None, target_bir_lowering=False, debug=False)
    xt_ext = nc.declare_dram_parameter("xt", [D, SQ_SHARD], cdt, isOutput=False)
    c_ext = nc.declare_dram_parameter("ctx", [SKV, D], cdt, isOutput=False)
    w_ext = nc.declare_dram_parameter("w", [D, 2 * D], cdt, isOutput=False)
    out_ext = nc.declare_dram_parameter("out", [SQ_SHARD, D], cdt, isOutput=True)

    NOG = 4  # out store groups
    OT = SQ_SHARD // 128 // NOG  # 4 tiles per group

    ctx_view = []
    row = 0
    for r in CTX_R:
        ctx_view.append(
            c_ext[row : row + 128 * r, :].rearrange("(p r) d -> p r d", p=128)
        )
        row += 128 * r
    out_view = [
        out_ext[g * 512 : (g + 1) * 512, :].rearrange("(p r) d -> p r d", p=128)
        for g in range(NOG)
    ]

    es = ExitStack()
    _n = [0]

    def sb(shape, dt, name=None):
        _n[0] += 1
        return es.enter_context(nc.sbuf_tensor(name or f"sb{_n[0]}", shape, dt))

    def pst(shape, dt, name=None):
        _n[0] += 1
        return es.enter_context(nc.psum_tensor(name or f"ps{_n[0]}", shape, dt))

    def sem(name):
        return es.enter_context(nc.semaphore(name))

    with es:
        w_sb = sb([D, 2 * D], cdt, "w_sb")
        cc = [sb([128, r, D], cdt, f"cc{i}") for i, r in enumerate(CTX_R)]
        xt_sb = sb([D, SQ_SHARD], cdt, "xt_sb")
        warm_sb = sb([D, D], cdt, "warm_sb")
        ga_bf = sb([D, D], cdt, "ga_bf")
        gb_bf = sb([D, D], cdt, "gb_bf")
        v_sb = sb([D, D], cdt, "v_sb")
        p2_sb = sb([D, D], cdt, "p2_sb")
        o_sb = [sb([128, OT, D], cdt, f"o_sb{i}") for i in range(NOG)]

        ga_ps = pst([128, 512], F32)  # b0
        gb_ps = pst([128, 512], F32)  # b1
        v_ps = pst([128, 512], F32)  # b2
        p2_ps = pst([128, 512], F32)  # b3
        o_ps = [pst([128, 512], F32) for _ in range(NOG)]  # b4-b7

        s_pe = sem("s_pe")
        s_dve = sem("s_dve")
        s_sc = sem("s_sc")
        s_q1 = sem("s_q1")
        s_q2 = sem("s_q2")
        s_q3 = sem("s_q3")
        s_st = sem("s_st")

        # chunk index -> (queue sem, threshold)
        chunk_gate = {}
        for qsem, chunks in ((s_q1, Q_SYNC), (s_q2, Q_SCAL)):
            for pos, ci in enumerate(chunks):
                chunk_gate[ci] = (qsem, 16 * (pos + 1))

        UT = w_sb[:, :D]
        W2 = w_sb[:, D:]

        with nc.Block() as block:

            @block.sync
            def _(sync):
                for i in Q_SYNC:
                    nc.sync.dma_start(cc[i][:], ctx_view[i]).then_inc(s_q1, 16)
                nc.sync.dma_start(
                    xt_sb[:, : SQ_SHARD // 2], xt_ext[:, : SQ_SHARD // 2]
                ).then_inc(s_q1, 16)
                nc.sync.wait_ge(s_dve, 4)  # o0 copied
                nc.sync.dma_start(out_view[0], o_sb[0][:]).then_inc(s_st, 16)
                nc.sync.wait_ge(s_sc, 2)  # o3 copied (scalar)
                nc.sync.dma_start(out_view[3], o_sb[3][:]).then_inc(s_st, 16)
                nc.sync.wait_ge(s_st, 64)

            @block.scalar
            def _(sc):
                for i in Q_SCAL:
                    nc.scalar.dma_start(cc[i][:], ctx_view[i]).then_inc(
                        s_q2, 16
                    )
                nc.scalar.wait_ge(s_pe, 32)  # G bank B done
                nc.scalar.copy(gb_bf[:], gb_ps[:, :128]).then_inc(s_sc, 1)
                nc.scalar.wait_ge(s_pe, 43)  # out group 1 done (b5)
                nc.scalar.copy(
                    o_sb[1][:].rearrange("p n d -> p (n d)"), o_ps[1][:]
                )
                nc.scalar.dma_start(out_view[1], o_sb[1][:]).then_inc(s_st, 16)
                nc.scalar.wait_ge(s_pe, 51)  # out group 3 done (b7)
                nc.scalar.copy(
                    o_sb[3][:].rearrange("p n d -> p (n d)"), o_ps[3][:]
                ).then_inc(s_sc, 1)
                nc.scalar.wait_ge(s_st, 64)

            @block.gpsimd
            def _(gp):
                nc.gpsimd.dma_start(w_sb[:], w_ext[:]).then_inc(s_q3, 16)
                nc.gpsimd.dma_start(
                    xt_sb[:, SQ_SHARD // 2 :], xt_ext[:, SQ_SHARD // 2 :]
                ).then_inc(s_q3, 16)
                nc.gpsimd.wait_ge(s_dve, 5)  # o2 copied
                nc.gpsimd.dma_start(out_view[2], o_sb[2][:]).then_inc(s_st, 16)
                nc.gpsimd.wait_ge(s_st, 64)

            @block.tensor
            def _(te):
                def fill(n):
                    # PE DVFS: full clock only after ~3us of continuous
                    # execution, and long stalls drop it back. Dummy matmuls
                    # on scratch data ramp the clock during DMA waits and
                    # hold it through the V/P2 chain gaps. Results land in
                    # b7, which out group 3 overwrites later.
                    for _ in range(n):
                        nc.tensor.matmul(
                            o_ps[3][:, :128],
                            warm_sb[:],
                            warm_sb[:],
                            start=True,
                            stop=True,
                        )

                fill(N_WARM)
                # G: alternate PSUM banks per matmul so consecutive matmuls
                # pipeline instead of serializing on one bank's accumulator
                m = 0
                n_mm = sum(CTX_R)
                for i in PE_CTX_ORDER:
                    qsem, thr = chunk_gate[i]
                    nc.tensor.wait_ge(qsem, thr)
                    for j in range(CTX_R[i]):
                        bank = ga_ps if m % 2 == 0 else gb_ps
                        nc.tensor.matmul(
                            bank[:, :128],
                            cc[i][:, j, :],
                            cc[i][:, j, :],
                            start=(m < 2),
                            stop=(m >= n_mm - 2),
                        ).then_inc(s_pe, 1)
                        m += 1
                # V = G W2 = (Ga + Gb) W2, accumulated from the two bank
                # copies (G symmetric -> bf16 bank copies are their own lhsT)
                fill(N_CHAIN)  # hold clock while vector/scalar cast banks
                nc.tensor.wait_ge(s_dve, 1)  # ga cast (vector)
                nc.tensor.wait_ge(s_sc, 1)  # gb cast (scalar)
                nc.tensor.matmul(
                    v_ps[:, :128], ga_bf[:], W2, start=True, stop=False
                ).then_inc(s_pe, 1)  # 33
                nc.tensor.matmul(
                    v_ps[:, :128], gb_bf[:], W2, start=False, stop=True
                ).then_inc(s_pe, 1)  # 34
                fill(N_CHAIN)  # hold clock while vector copies V
                # P2 = U V
                nc.tensor.wait_ge(s_dve, 2)  # v copied
                nc.tensor.matmul(
                    p2_ps[:, :128], UT, v_sb[:], start=True, stop=True
                ).then_inc(s_pe, 1)  # 35
                fill(N_CHAIN)  # hold clock while vector copies P2
                # out = X P2: lhsT slices straight from host-transposed X
                nc.tensor.wait_ge(s_dve, 3)  # p2 copied
                nc.tensor.wait_ge(s_q1, 48)  # xt half 0
                for k in range(16):  # 36-51
                    if k == 8:
                        nc.tensor.wait_ge(s_q3, 32)  # xt half 1
                    nc.tensor.matmul(
                        o_ps[k // 4][:, (k % 4) * D : (k % 4 + 1) * D],
                        xt_sb[:, k * 128 : (k + 1) * 128],
                        p2_sb[:],
                        start=True,
                        stop=True,
                    ).then_inc(s_pe, 1)

            @block.vector
            def _(ve):
                nc.vector.wait_ge(s_pe, 31)  # G bank A done (last even m=30)
                nc.vector.tensor_copy(ga_bf[:], ga_ps[:, :128]).then_inc(
                    s_dve, 1
                )
                nc.vector.wait_ge(s_pe, 34)
                nc.vector.tensor_copy(v_sb[:], v_ps[:, :128]).then_inc(
                    s_dve, 1
                )
                nc.vector.wait_ge(s_pe, 35)
                nc.vector.tensor_copy(p2_sb[:], p2_ps[:, :128]).then_inc(
                    s_dve, 1
                )
                nc.vector.wait_ge(s_pe, 39)  # out group 0 done (b4)
                nc.vector.tensor_copy(
                    o_sb[0][:].rearrange("p n d -> p (n d)"), o_ps[0][:]
                ).then_inc(s_dve, 1)
                nc.vector.wait_ge(s_pe, 47)  # out group 2 done (b6)
                nc.vector.tensor_copy(
                    o_sb[2][:].rearrange("p n d -> p (n d)"), o_ps[2][:]
                ).then_inc(s_dve, 1)

    nc.compile()
    return nc


def _get_nc():
    if "nc" not in _CACHE:
        _CACHE["nc"] = build_raw()
    return _CACHE["nc"]


def _prep_in_maps(inputs: dict):
    bf16 = ml_dtypes.bfloat16
    context = np.ascontiguousarray(inputs["context"]).astype(bf16)
    X = np.ascontiguousarray(inputs["X"]).astype(np.float32)
    Wq = np.ascontiguousarray(inputs["Wq"]).astype(np.float32)
    Wk = np.ascontiguousarray(inputs["Wk"]).astype(np.float32)
    Wv = np.ascontiguousarray(inputs["Wv"]).astype(np.float32)

    UT = Wk.T @ Wq  # (Wq^T Wk)^T
    W2 = SCALE * Wv.T
    w_host = np.ascontiguousarray(np.concatenate([UT, W2], axis=1).astype(bf16))

    Xb = X.astype(bf16)
    in_maps = []
    for c in range(N_CORES):
        b, h = divmod(c, 2)
        xt = np.ascontiguousarray(Xb[b, h * SQ_SHARD : (h + 1) * SQ_SHARD, :].T)
        in_maps.append(
            {"xt": xt, "ctx": np.ascontiguousarray(context[b]), "w": w_host}
        )
    return in_maps


def _unpermute(dev: np.ndarray) -> np.ndarray:
    # dev row g*512 + p*4 + j holds true row g*512 + j*128 + p
    return dev.reshape(4, 128, 4, D).transpose(0, 2, 1, 3).reshape(SQ_SHARD, D)


def _run(inputs: dict, trace: bool = False, **kw):
    in_maps = _prep_in_maps(inputs)
    nc = _get_nc()
    res = run_bass_kernel_spmd(
        nc, in_maps, core_ids=list(range(N_CORES)), trace=trace, **kw
    )
    out = np.empty((B, SQ, D), dtype=np.float32)
    for c in range(N_CORES):
        b, h = divmod(c, 2)
        out[b, h * SQ_SHARD : (h + 1) * SQ_SHARD, :] = _unpermute(
            res.results[c]["out"]
        ).astype(np.float32)
    return out, res


def kernel(**inputs: np.ndarray) -> np.ndarray:
    if os.environ.get("BASS_TRACE"):
        _install_axon_ntff_shim()
    try:
        out, _ = _run(inputs, trace=False)
    except Exception:
        # transient NRT device errors have been observed once across many
        # runs; one retry on a fresh execution
        out, _ = _run(inputs, trace=False)
    return out


if __name__ == "__main__":
    rng = np.random.default_rng(0)
    ins = {
        "context": rng.standard_normal((B, SKV, D)).astype(np.float32),
        "X": rng.standard_normal((B, SQ, D)).astype(np.float32),
        "Wq": (rng.standard_normal((D, D)) / np.sqrt(D)).astype(np.float32),
        "Wk": (rng.standard_normal((D, D)) / np.sqrt(D)).astype(np.float32),
        "Wv": (rng.standard_normal((D, D)) / np.sqrt(D)).astype(np.float32),
    }
    got = kernel(**ins)
    q = ins["X"] @ ins["Wq"].T
    k = ins["context"] @ ins["Wk"].T
    v = ins["context"] @ ins["Wv"].T
    w = np.einsum("bse,bte->bst", q, k) * SCALE
    want = np.einsum("bst,bte->bse", w, v)
    rel = np.linalg.norm(got - want) / np.linalg.norm(want)
    print("rel err vs numpy:", rel)


# revision 21
# speedup vs baseline: 1.1657x; 1.1339x over previous
"""Cross-attention without softmax on 8 trn2 NeuronCores.

Reference computes out = (X Wq^T) (C Wk^T)^T (C Wv^T) * D^-0.5 per batch.
With no softmax the product reassociates:

    out_b = X_b @ P2_b,  P2_b = U G_b W2,  U = Wq^T Wk,
    G_b = C_b^T C_b,     W2 = D^-0.5 Wv^T

U and W2 are weight-only and precomputed on the host. The device computes
G (32 accumulating 128x128 matmuls alternating between two PSUM banks),
then V = G W2 as two accumulating matmuls off the two banks (G is
symmetric so the bank copies are their own lhsT), P2 = U V, and finally
out = X P2 as 16 matmuls whose lhsT slices come straight from a
host-pre-transposed X — no on-device transposes, no Q' intermediate.

Two hardware quirks shape the schedule:
 1. PE DVFS: the tensor engine reaches its 2.4GHz p-state only after
    ~3us of continuous execution and long stalls drop it back, so the
    tensor program front-runs the first ctx DMA with dummy warmup
    matmuls and keeps short filler bursts in the V/P2 chain gaps; real
    matmuls then issue at ~56ns/tile instead of ~107ns.
 2. Each dma_start costs ~1.7us of queue occupancy (128 descriptors,
    one per partition, ~13ns each) nearly independent of size, so the
    kernel moves everything in SIX fat DMAs: one 512KB ctx half per HW
    queue (4KB/partition descriptors), w + the whole 512KB xt on the
    gpsimd SW queue, and two 256KB output stores (2KB/partition).

Sharding: batch (4) x query-half (2) -> 8 cores; each core redundantly
computes its batch's G (no collectives). I/O is pre-cast to bf16 on the
host; accumulation stays fp32 in PSUM. ctx tiles use the permuted
grouping (partition p holds rows base + p*16 + j); G's row-sum is
invariant to the permutation. Out rows are stored in a device-friendly
permuted order (dev row h*1024+p*8+j holds true row h*1024+j*128+p)
and un-permuted on the host.
"""

import os
import sys
import types

import numpy as np

_TRN_REPO = "/opt/trn_rl_repo"
if _TRN_REPO not in sys.path and not any("trn_rl_repo" in p for p in sys.path):
    sys.path.insert(0, _TRN_REPO)

import ml_dtypes  # noqa: E402

import concourse.bass as bass  # noqa: E402
import concourse.mybir as mybir  # noqa: E402
from concourse import bacc  # noqa: E402
from concourse.bass_utils import run_bass_kernel_spmd  # noqa: E402

B, SQ, SKV, D = 4, 4096, 4096, 128
N_CORES = 8
SQ_SHARD = SQ // (N_CORES // B)  # 2048
SCALE = float(D) ** -0.5
F32 = mybir.dt.float32
BF16 = mybir.dt.bfloat16

_CACHE: dict = {}


def _install_axon_ntff_shim():
    try:
        import antenv.axon_hooks  # noqa: F401

        return
    except Exception:
        pass
    try:
        from trn_agent_boot.trn_boot import _ntff_profile_via_ctypes

        import antenv

        hook = _ntff_profile_via_ctypes("/opt/axon/libaxon_pjrt.so")
        mod = types.ModuleType("antenv.axon_hooks")
        mod._hook = hook
        mod.get_axon_ntff_profile_hook = lambda: mod._hook

        def _set(h):
            mod._hook = h

        mod.set_axon_ntff_profile_hook = _set
        antenv.axon_hooks = mod
        sys.modules["antenv.axon_hooks"] = mod
    except Exception:
        pass

    try:
        import concourse.bass_utils as bu

        bu.upload_artifacts = lambda tmpdir: f"file://{tmpdir}"
    except Exception:
        pass


# PE clock-ramp tuning (see fill() in build_raw)
N_WARM = int(os.environ.get("KERNEL_WARMUP", "28"))
N_CHAIN = int(os.environ.get("KERNEL_CHAINFILL", "3"))

RC = 16  # rows per partition in each ctx half (4KB descriptors)


def build_raw():
    """Hand-scheduled raw-bass kernel. Per-core inputs:
    xt (128, 2048) = X-shard transposed, ctx (4096, 128),
    w (128, 256) = [U^T | W2]; output out (2048, 128), permuted rows.

    Queues (FIFO per queue; one semaphore per queue, +16 per DMA):
      sync  HW (s_q1): ctx half 0; later store A (out rows 0-1023)
      scalar HW (s_q2): ctx half 1; later store B (out rows 1024-2047)
      gpsimd SW (s_q3): w (16), xt (32)

    Cumulative schedules (value after the op):
      PE (s_pe): G 1-32 (bank A even m, bank B odd m), V 33-34, P2 35,
                 out 36-51 (4 groups of 4, banks b4-b7)
      vector (s_dve): gaA cast 1, v 2, p2 3, oA[0:4] 4, oB[4:8] 5
      scalar (s_sc): gbB cast 1, oA[4:8] 2
      gpsimd (s_q3): ... o2 copy -> 48 (oB[0:4])

    PSUM banks: b0/b1 = G even/odd accumulators; b2 = V; b3 = P2;
    b4-b7 = out groups 0-3 (b7 also absorbs warmup/filler matmuls).
    """
    from contextlib import ExitStack

    cdt = BF16

    nc = bacc.Bacc(None, target_bir_lowering=False, debug=False)
    xt_ext = nc.declare_dram_parameter("xt", [D, SQ_SHARD], cdt, isOutput=False)
    c_ext = nc.declare_dram_parameter("ctx", [SKV, D], cdt, isOutput=False)
    w_ext = nc.declare_dram_parameter("w", [D, 2 * D], cdt, isOutput=False)
    out_ext = nc.declare_dram_parameter("out", [SQ_SHARD, D], cdt, isOutput=True)

    ctx_view = [
        c_ext[h * 2048 : (h + 1) * 2048, :].rearrange("(p r) d -> p r d", p=128)
        for h in range(2)
    ]
    out_view = [
        out_ext[h * 1024 : (h + 1) * 1024, :].rearrange("(p r) d -> p r d", p=128)
        for h in range(2)
    ]

    es = ExitStack()
    _n = [0]

    def sb(shape, dt, name=None):
        _n[0] += 1
        return es.enter_context(nc.sbuf_tensor(name or f"sb{_n[0]}", shape, dt))

    def pst(shape, dt, name=None):
        _n[0] += 1
        return es.enter_context(nc.psum_tensor(name or f"ps{_n[0]}", shape, dt))

    def sem(name):
        return es.enter_context(nc.semaphore(name))

    with es:
        w_sb = sb([D, 2 * D], cdt, "w_sb")
        cc = [sb([128, RC, D], cdt, f"cc{i}") for i in range(2)]
        xt_sb = sb([D, SQ_SHARD], cdt, "xt_sb")
        warm_sb = sb([D, D], cdt, "warm_sb")
        ga_bf = sb([D, D], cdt, "ga_bf")
        gb_bf = sb([D, D], cdt, "gb_bf")
        v_sb = sb([D, D], cdt, "v_sb")
        p2_sb = sb([D, D], cdt, "p2_sb")
        oA_sb = sb([128, 8, D], cdt, "oA_sb")
        oB_sb = sb([128, 8, D], cdt, "oB_sb")

        ga_ps = pst([128, 512], F32)  # b0
        gb_ps = pst([128, 512], F32)  # b1
        v_ps = pst([128, 512], F32)  # b2
        p2_ps = pst([128, 512], F32)  # b3
        o_ps = [pst([128, 512], F32) for _ in range(4)]  # b4-b7

        s_pe = sem("s_pe")
        s_dve = sem("s_dve")
        s_sc = sem("s_sc")
        s_q1 = sem("s_q1")
        s_q2 = sem("s_q2")
        s_q3 = sem("s_q3")
        s_st = sem("s_st")

        UT = w_sb[:, :D]
        W2 = w_sb[:, D:]

        with nc.Block() as block:

            @block.sync
            def _(sync):
                nc.sync.dma_start(cc[0][:], ctx_view[0]).then_inc(s_q1, 16)
                nc.sync.wait_ge(s_dve, 4)  # oA[0:4] copied (vector)
                nc.sync.wait_ge(s_sc, 2)  # oA[4:8] copied (scalar)
                nc.sync.dma_start(out_view[0], oA_sb[:]).then_inc(s_st, 16)
                nc.sync.wait_ge(s_st, 32)

            @block.scalar
            def _(sc):
                nc.scalar.dma_start(cc[1][:], ctx_view[1]).then_inc(s_q2, 16)
                nc.scalar.wait_ge(s_pe, 32)  # G bank B done
                nc.scalar.copy(gb_bf[:], gb_ps[:, :128]).then_inc(s_sc, 1)
                nc.scalar.wait_ge(s_pe, 43)  # out group 1 done (b5)
                nc.scalar.copy(
                    oA_sb[:, 4:8, :].rearrange("p n d -> p (n d)"), o_ps[1][:]
                ).then_inc(s_sc, 1)
                nc.scalar.wait_ge(s_pe, 51)  # out group 3 done (b7)
                nc.scalar.copy(
                    oB_sb[:, 4:8, :].rearrange("p n d -> p (n d)"), o_ps[3][:]
                )
                nc.scalar.wait_ge(s_dve, 5)  # oB[0:4] copied (vector)
                nc.scalar.dma_start(out_view[1], oB_sb[:]).then_inc(s_st, 16)
                nc.scalar.wait_ge(s_st, 32)

            @block.gpsimd
            def _(gp):
                nc.gpsimd.dma_start(w_sb[:], w_ext[:]).then_inc(s_q3, 16)
                nc.gpsimd.dma_start(xt_sb[:], xt_ext[:]).then_inc(s_q3, 16)
                nc.gpsimd.wait_ge(s_st, 32)

            @block.tensor
            def _(te):
                def fill(n):
                    # PE DVFS warmup / clock-hold (results discarded; b7 is
                    # overwritten by out group 3 later)
                    for _ in range(n):
                        nc.tensor.matmul(
                            o_ps[3][:, :128],
                            warm_sb[:],
                            warm_sb[:],
                            start=True,
                            stop=True,
                        )

                fill(N_WARM)
                # G: alternate PSUM banks per matmul so consecutive matmuls
                # pipeline instead of serializing on one bank's accumulator
                m = 0
                for h in range(2):
                    nc.tensor.wait_ge((s_q1, s_q2)[h], 16)
                    for j in range(RC):
                        bank = ga_ps if m % 2 == 0 else gb_ps
                        nc.tensor.matmul(
                            bank[:, :128],
                            cc[h][:, j, :],
                            cc[h][:, j, :],
                            start=(m < 2),
                            stop=(m >= 2 * RC - 2),
                        ).then_inc(s_pe, 1)
                        m += 1
                # V = G W2 = (Ga + Gb) W2, accumulated from the two bank
                # copies (G symmetric -> bf16 bank copies are their own lhsT)
                fill(N_CHAIN)  # hold clock while vector/scalar cast banks
                nc.tensor.wait_ge(s_dve, 1)  # ga cast (vector)
                nc.tensor.wait_ge(s_sc, 1)  # gb cast (scalar)
                nc.tensor.matmul(
                    v_ps[:, :128], ga_bf[:], W2, start=True, stop=False
                ).then_inc(s_pe, 1)  # 33
                nc.tensor.matmul(
                    v_ps[:, :128], gb_bf[:], W2, start=False, stop=True
                ).then_inc(s_pe, 1)  # 34
                fill(N_CHAIN)  # hold clock while vector copies V
                # P2 = U V
                nc.tensor.wait_ge(s_dve, 2)  # v copied
                nc.tensor.matmul(
                    p2_ps[:, :128], UT, v_sb[:], start=True, stop=True
                ).then_inc(s_pe, 1)  # 35
                fill(N_CHAIN)  # hold clock while vector copies P2
                # out = X P2: lhsT slices straight from host-transposed X
                nc.tensor.wait_ge(s_dve, 3)  # p2 copied
                nc.tensor.wait_ge(s_q3, 32)  # xt loaded
                for k in range(16):  # 36-51
                    nc.tensor.matmul(
                        o_ps[k // 4][:, (k % 4) * D : (k % 4 + 1) * D],
                        xt_sb[:, k * 128 : (k + 1) * 128],
                        p2_sb[:],
                        start=True,
                        stop=True,
                    ).then_inc(s_pe, 1)

            @block.vector
            def _(ve):
                nc.vector.wait_ge(s_pe, 31)  # G bank A done (last even m=30)
                nc.vector.tensor_copy(ga_bf[:], ga_ps[:, :128]).then_inc(
                    s_dve, 1
                )
                nc.vector.wait_ge(s_pe, 34)
                nc.vector.tensor_copy(v_sb[:], v_ps[:, :128]).then_inc(
                    s_dve, 1
                )
                nc.vector.wait_ge(s_pe, 35)
                nc.vector.tensor_copy(p2_sb[:], p2_ps[:, :128]).then_inc(
                    s_dve, 1
                )
                nc.vector.wait_ge(s_pe, 39)  # out group 0 done (b4)
                nc.vector.tensor_copy(
                    oA_sb[:, 0:4, :].rearrange("p n d -> p (n d)"), o_ps[0][:]
                ).then_inc(s_dve, 1)
                nc.vector.wait_ge(s_pe, 47)  # out group 2 done (b6)
                nc.vector.tensor_copy(
                    oB_sb[:, 0:4, :].rearrange("p n d -> p (n d)"), o_ps[2][:]
                ).then_inc(s_dve, 1)

    nc.compile()
    return nc


def _get_nc():
    if "nc" not in _CACHE:
        _CACHE["nc"] = build_raw()
    return _CACHE["nc"]


def _prep_in_maps(inputs: dict):
    bf16 = ml_dtypes.bfloat16
    context = np.ascontiguousarray(inputs["context"]).astype(bf16)
    X = np.ascontiguousarray(inputs["X"]).astype(np.float32)
    Wq = np.ascontiguousarray(inputs["Wq"]).astype(np.float32)
    Wk = np.ascontiguousarray(inputs["Wk"]).astype(np.float32)
    Wv = np.ascontiguousarray(inputs["Wv"]).astype(np.float32)

    UT = Wk.T @ Wq  # (Wq^T Wk)^T
    W2 = SCALE * Wv.T
    w_host = np.ascontiguousarray(np.concatenate([UT, W2], axis=1).astype(bf16))

    Xb = X.astype(bf16)
    in_maps = []
    for c in range(N_CORES):
        b, h = divmod(c, 2)
        xt = np.ascontiguousarray(Xb[b, h * SQ_SHARD : (h + 1) * SQ_SHARD, :].T)
        in_maps.append(
            {"xt": xt, "ctx": np.ascontiguousarray(context[b]), "w": w_host}
        )
    return in_maps


def _unpermute(dev: np.ndarray) -> np.ndarray:
    # dev row h*1024 + p*8 + j holds true row h*1024 + j*128 + p
    return dev.reshape(2, 128, 8, D).transpose(0, 2, 1, 3).reshape(SQ_SHARD, D)


def _run(inputs: dict, trace: bool = False, **kw):
    in_maps = _prep_in_maps(inputs)
    nc = _get_nc()
    res = run_bass_kernel_spmd(
        nc, in_maps, core_ids=list(range(N_CORES)), trace=trace, **kw
    )
    out = np.empty((B, SQ, D), dtype=np.float32)
    for c in range(N_CORES):
        b, h = divmod(c, 2)
        out[b, h * SQ_SHARD : (h + 1) * SQ_SHARD, :] = _unpermute(
            res.results[c]["out"]
        ).astype(np.float32)
    return out, res


def kernel(**inputs: np.ndarray) -> np.ndarray:
    if os.environ.get("BASS_TRACE"):
        _install_axon_ntff_shim()
    try:
        out, _ = _run(inputs, trace=False)
    except Exception:
        # transient NRT device errors have been observed once across many
        # runs; one retry on a fresh execution
        out, _ = _run(inputs, trace=False)
    return out


if __name__ == "__main__":
    rng = np.random.default_rng(0)
    ins = {
        "context": rng.standard_normal((B, SKV, D)).astype(np.float32),
        "X": rng.standard_normal((B, SQ, D)).astype(np.float32),
        "Wq": (rng.standard_normal((D, D)) / np.sqrt(D)).astype(np.float32),
        "Wk": (rng.standard_normal((D, D)) / np.sqrt(D)).astype(np.float32),
        "Wv": (rng.standard_normal((D, D)) / np.sqrt(D)).astype(np.float32),
    }
    got = kernel(**ins)
    q = ins["X"] @ ins["Wq"].T
    k = ins["context"] @ ins["Wk"].T
    v = ins["context"] @ ins["Wv"].T
    w = np.einsum("bse,bte->bst", q, k) * SCALE
    want = np.einsum("bst,bte->bse", w, v)
    rel = np.linalg.norm(got - want) / np.linalg.norm(want)
    print("rel err vs numpy:", rel)


# revision 27
# speedup vs baseline: 1.1708x; 1.0043x over previous
"""Cross-attention without softmax on 8 trn2 NeuronCores.

Reference computes out = (X Wq^T) (C Wk^T)^T (C Wv^T) * D^-0.5 per batch.
With no softmax the product reassociates:

    out_b = X_b @ P2_b,  P2_b = U G_b W2,  U = Wq^T Wk,
    G_b = C_b^T C_b,     W2 = D^-0.5 Wv^T

U and W2 are weight-only and precomputed on the host. The device computes
G (32 accumulating 128x128 matmuls alternating between two PSUM banks),
then V = G W2 as two accumulating matmuls off the two banks (G is
symmetric so the bank copies are their own lhsT), P2 = U V, and finally
out = X P2 as 16 matmuls whose lhsT slices come straight from a
host-pre-transposed X — no on-device transposes, no Q' intermediate.

Two hardware quirks shape the schedule:
 1. PE DVFS: the tensor engine reaches its 2.4GHz p-state only after
    ~3us of continuous execution and long stalls drop it back, so the
    tensor program front-runs the first ctx DMA with dummy warmup
    matmuls and keeps short filler bursts in the V/P2 chain gaps; real
    matmuls then issue at ~56ns/tile instead of ~107ns.
 2. Each dma_start costs ~1.7us of queue occupancy (128 descriptors,
    one per partition, ~13ns each) nearly independent of size, so the
    kernel moves everything in SIX fat DMAs: one 512KB ctx half per HW
    queue (4KB/partition descriptors), w + the whole 512KB xt on the
    gpsimd SW queue, and two 256KB output stores (2KB/partition).

Sharding: batch (4) x query-half (2) -> 8 cores; each core redundantly
computes its batch's G (no collectives). I/O is pre-cast to bf16 on the
host; accumulation stays fp32 in PSUM. ctx tiles use the permuted
grouping (partition p holds rows base + p*16 + j); G's row-sum is
invariant to the permutation. Out rows are stored in a device-friendly
permuted order (dev row h*1024+p*8+j holds true row h*1024+j*128+p)
and un-permuted on the host.
"""

import os
import sys
import types

import numpy as np

_TRN_REPO = "/opt/trn_rl_repo"
if _TRN_REPO not in sys.path and not any("trn_rl_repo" in p for p in sys.path):
    sys.path.insert(0, _TRN_REPO)

import ml_dtypes  # noqa: E402

import concourse.bass as bass  # noqa: E402
import concourse.mybir as mybir  # noqa: E402
from concourse import bacc  # noqa: E402
from concourse.bass_utils import run_bass_kernel_spmd  # noqa: E402

B, SQ, SKV, D = 4, 4096, 4096, 128
N_CORES = 8
SQ_SHARD = SQ // (N_CORES // B)  # 2048
SCALE = float(D) ** -0.5
F32 = mybir.dt.float32
BF16 = mybir.dt.bfloat16

_CACHE: dict = {}


def _install_axon_ntff_shim():
    try:
        import antenv.axon_hooks  # noqa: F401

        return
    except Exception:
        pass
    try:
        from trn_agent_boot.trn_boot import _ntff_profile_via_ctypes

        import antenv

        hook = _ntff_profile_via_ctypes("/opt/axon/libaxon_pjrt.so")
        mod = types.ModuleType("antenv.axon_hooks")
        mod._hook = hook
        mod.get_axon_ntff_profile_hook = lambda: mod._hook

        def _set(h):
            mod._hook = h

        mod.set_axon_ntff_profile_hook = _set
        antenv.axon_hooks = mod
        sys.modules["antenv.axon_hooks"] = mod
    except Exception:
        pass

    try:
        import concourse.bass_utils as bu

        bu.upload_artifacts = lambda tmpdir: f"file://{tmpdir}"
    except Exception:
        pass


# PE clock-ramp tuning (see fill() in build_raw)
N_WARM = int(os.environ.get("KERNEL_WARMUP", "28"))
N_CHAIN = int(os.environ.get("KERNEL_CHAINFILL", "3"))

RC = 8  # rows per partition in each ctx quarter (2KB descriptors)


def build_raw():
    """Hand-scheduled raw-bass kernel. Per-core inputs:
    xt (128, 2048) = X-shard transposed, ctx (4096, 128),
    w (128, 256) = [U^T | W2]; output out (2048, 128), permuted rows.

    Queues (FIFO per queue; one semaphore per queue, +16 per DMA):
      sync  HW (s_q1): ctx half 0; later store A (out rows 0-1023)
      scalar HW (s_q2): ctx half 1; later store B (out rows 1024-2047)
      gpsimd SW (s_q3): w (16), xt (32)

    Cumulative schedules (value after the op):
      PE (s_pe): G 1-32 (bank A even m, bank B odd m), V 33-34, P2 35,
                 out 36-51 (4 groups of 4, banks b4-b7)
      vector (s_dve): gaA cast 1, v 2, p2 3, oA[0:4] 4, oB[4:8] 5
      scalar (s_sc): gbB cast 1, oA[4:8] 2
      gpsimd (s_q3): ... o2 copy -> 48 (oB[0:4])

    PSUM banks: b0/b1 = G even/odd accumulators; b2 = V; b3 = P2;
    b4-b7 = out groups 0-3 (b7 also absorbs warmup/filler matmuls).
    """
    from contextlib import ExitStack

    cdt = BF16

    nc = bacc.Bacc(None, target_bir_lowering=False, debug=False)
    xt_ext = nc.declare_dram_parameter("xt", [D, SQ_SHARD], cdt, isOutput=False)
    c_ext = nc.declare_dram_parameter("ctx", [SKV, D], cdt, isOutput=False)
    w_ext = nc.declare_dram_parameter("w", [D, 2 * D], cdt, isOutput=False)
    out_ext = nc.declare_dram_parameter("out", [SQ_SHARD, D], cdt, isOutput=True)

    ctx_view = [
        c_ext[q * 1024 : (q + 1) * 1024, :].rearrange("(p r) d -> p r d", p=128)
        for q in range(4)
    ]
    out_view = [
        out_ext[h * 1024 : (h + 1) * 1024, :].rearrange("(p r) d -> p r d", p=128)
        for h in range(2)
    ]

    es = ExitStack()
    _n = [0]

    def sb(shape, dt, name=None):
        _n[0] += 1
        return es.enter_context(nc.sbuf_tensor(name or f"sb{_n[0]}", shape, dt))

    def pst(shape, dt, name=None):
        _n[0] += 1
        return es.enter_context(nc.psum_tensor(name or f"ps{_n[0]}", shape, dt))

    def sem(name):
        return es.enter_context(nc.semaphore(name))

    with es:
        w_sb = sb([D, 2 * D], cdt, "w_sb")
        cc = [sb([128, RC, D], cdt, f"cc{i}") for i in range(4)]
        xt_sb = sb([D, SQ_SHARD], cdt, "xt_sb")
        warm_sb = sb([D, D], cdt, "warm_sb")
        ga_bf = sb([D, D], cdt, "ga_bf")
        gb_bf = sb([D, D], cdt, "gb_bf")
        v_sb = sb([D, D], cdt, "v_sb")
        p2_sb = sb([D, D], cdt, "p2_sb")
        oA_sb = sb([128, 8, D], cdt, "oA_sb")
        oB_sb = sb([128, 8, D], cdt, "oB_sb")

        ga_ps = pst([128, 512], F32)  # b0
        gb_ps = pst([128, 512], F32)  # b1
        v_ps = pst([128, 512], F32)  # b2
        p2_ps = pst([128, 512], F32)  # b3
        o_ps = [pst([128, 512], F32) for _ in range(4)]  # b4-b7

        s_pe = sem("s_pe")
        s_dve = sem("s_dve")
        s_sc = sem("s_sc")
        s_q1 = sem("s_q1")
        s_q2 = sem("s_q2")
        s_q3 = sem("s_q3")
        s_st = sem("s_st")

        UT = w_sb[:, :D]
        W2 = w_sb[:, D:]

        with nc.Block() as block:

            @block.sync
            def _(sync):
                nc.sync.dma_start(cc[0][:], ctx_view[0]).then_inc(s_q1, 16)
                nc.sync.dma_start(cc[2][:], ctx_view[2]).then_inc(s_q1, 16)
                nc.sync.wait_ge(s_dve, 4)  # oA[0:4] copied (vector)
                nc.sync.wait_ge(s_sc, 2)  # oA[4:8] copied (scalar)
                nc.sync.dma_start(out_view[0], oA_sb[:]).then_inc(s_st, 16)
                nc.sync.wait_ge(s_st, 32)

            @block.scalar
            def _(sc):
                nc.scalar.dma_start(cc[1][:], ctx_view[1]).then_inc(s_q2, 16)
                nc.scalar.dma_start(cc[3][:], ctx_view[3]).then_inc(s_q2, 16)
                nc.scalar.wait_ge(s_pe, 32)  # G bank B done
                nc.scalar.copy(gb_bf[:], gb_ps[:, :128]).then_inc(s_sc, 1)
                nc.scalar.wait_ge(s_pe, 43)  # out group 1 done (b5)
                nc.scalar.copy(
                    oA_sb[:, 4:8, :].rearrange("p n d -> p (n d)"), o_ps[1][:]
                ).then_inc(s_sc, 1)
                nc.scalar.wait_ge(s_pe, 51)  # out group 3 done (b7)
                nc.scalar.copy(
                    oB_sb[:, 4:8, :].rearrange("p n d -> p (n d)"), o_ps[3][:]
                )
                nc.scalar.wait_ge(s_dve, 5)  # oB[0:4] copied (vector)
                nc.scalar.dma_start(out_view[1], oB_sb[:]).then_inc(s_st, 16)
                nc.scalar.wait_ge(s_st, 32)

            @block.gpsimd
            def _(gp):
                nc.gpsimd.dma_start(w_sb[:], w_ext[:]).then_inc(s_q3, 16)
                nc.gpsimd.dma_start(xt_sb[:], xt_ext[:]).then_inc(s_q3, 16)
                nc.gpsimd.wait_ge(s_st, 32)

            @block.tensor
            def _(te):
                def fill(n):
                    # PE DVFS warmup / clock-hold (results discarded; b7 is
                    # overwritten by out group 3 later)
                    for _ in range(n):
                        nc.tensor.matmul(
                            o_ps[3][:, :128],
                            warm_sb[:],
                            warm_sb[:],
                            start=True,
                            stop=True,
                        )

                fill(N_WARM)
                # G: alternate PSUM banks per matmul so consecutive matmuls
                # pipeline instead of serializing on one bank's accumulator
                m = 0
                n_mm = 4 * RC
                gates = ((s_q1, 16), (s_q2, 16), (s_q1, 32), (s_q2, 32))
                for q in range(4):
                    nc.tensor.wait_ge(*gates[q])
                    for j in range(RC):
                        bank = ga_ps if m % 2 == 0 else gb_ps
                        nc.tensor.matmul(
                            bank[:, :128],
                            cc[q][:, j, :],
                            cc[q][:, j, :],
                            start=(m < 2),
                            stop=(m >= n_mm - 2),
                        ).then_inc(s_pe, 1)
                        m += 1
                # V = G W2 = (Ga + Gb) W2, accumulated from the two bank
                # copies (G symmetric -> bf16 bank copies are their own lhsT)
                fill(N_CHAIN)  # hold clock while vector/scalar cast banks
                nc.tensor.wait_ge(s_dve, 1)  # ga cast (vector)
                nc.tensor.wait_ge(s_sc, 1)  # gb cast (scalar)
                nc.tensor.matmul(
                    v_ps[:, :128], ga_bf[:], W2, start=True, stop=False
                ).then_inc(s_pe, 1)  # 33
                nc.tensor.matmul(
                    v_ps[:, :128], gb_bf[:], W2, start=False, stop=True
                ).then_inc(s_pe, 1)  # 34
                fill(N_CHAIN)  # hold clock while vector copies V
                # P2 = U V
                nc.tensor.wait_ge(s_dve, 2)  # v copied
                nc.tensor.matmul(
                    p2_ps[:, :128], UT, v_sb[:], start=True, stop=True
                ).then_inc(s_pe, 1)  # 35
                fill(N_CHAIN)  # hold clock while vector copies P2
                # out = X P2: lhsT slices straight from host-transposed X
                nc.tensor.wait_ge(s_dve, 3)  # p2 copied
                nc.tensor.wait_ge(s_q3, 32)  # xt loaded
                for k in range(16):  # 36-51
                    nc.tensor.matmul(
                        o_ps[k // 4][:, (k % 4) * D : (k % 4 + 1) * D],
                        xt_sb[:, k * 128 : (k + 1) * 128],
                        p2_sb[:],
                        start=True,
                        stop=True,
                    ).then_inc(s_pe, 1)

            @block.vector
            def _(ve):
                nc.vector.wait_ge(s_pe, 31)  # G bank A done (last even m=30)
                nc.vector.tensor_copy(ga_bf[:], ga_ps[:, :128]).then_inc(
                    s_dve, 1
                )
                nc.vector.wait_ge(s_pe, 34)
                nc.vector.tensor_copy(v_sb[:], v_ps[:, :128]).then_inc(
                    s_dve, 1
                )
                nc.vector.wait_ge(s_pe, 35)
                nc.vector.tensor_copy(p2_sb[:], p2_ps[:, :128]).then_inc(
                    s_dve, 1
                )
                nc.vector.wait_ge(s_pe, 39)  # out group 0 done (b4)
                nc.vector.tensor_copy(
                    oA_sb[:, 0:4, :].rearrange("p n d -> p (n d)"), o_ps[0][:]
                ).then_inc(s_dve, 1)
                nc.vector.wait_ge(s_pe, 47)  # out group 2 done (b6)
                nc.vector.tensor_copy(
                    oB_sb[:, 0:4, :].rearrange("p n d -> p (n d)"), o_ps[2][:]
                ).then_inc(s_dve, 1)

    nc.compile()
    return nc


def _get_nc():
    if "nc" not in _CACHE:
        _CACHE["nc"] = build_raw()
    return _CACHE["nc"]


def _prep_in_maps(inputs: dict):
    bf16 = ml_dtypes.bfloat16
    context = np.ascontiguousarray(inputs["context"]).astype(bf16)
    X = np.ascontiguousarray(inputs["X"]).astype(np.float32)
    Wq = np.ascontiguousarray(inputs["Wq"]).astype(np.float32)
    Wk = np.ascontiguousarray(inputs["Wk"]).astype(np.float32)
    Wv = np.ascontiguousarray(inputs["Wv"]).astype(np.float32)

    UT = Wk.T @ Wq  # (Wq^T Wk)^T
    W2 = SCALE * Wv.T
    w_host = np.ascontiguousarray(np.concatenate([UT, W2], axis=1).astype(bf16))

    Xb = X.astype(bf16)
    in_maps = []
    for c in range(N_CORES):
        b, h = divmod(c, 2)
        xt = np.ascontiguousarray(Xb[b, h * SQ_SHARD : (h + 1) * SQ_SHARD, :].T)
        in_maps.append(
            {"xt": xt, "ctx": np.ascontiguousarray(context[b]), "w": w_host}
        )
    return in_maps


def _unpermute(dev: np.ndarray) -> np.ndarray:
    # dev row h*1024 + p*8 + j holds true row h*1024 + j*128 + p
    return dev.reshape(2, 128, 8, D).transpose(0, 2, 1, 3).reshape(SQ_SHARD, D)


def _run(inputs: dict, trace: bool = False, **kw):
    in_maps = _prep_in_maps(inputs)
    nc = _get_nc()
    res = run_bass_kernel_spmd(
        nc, in_maps, core_ids=list(range(N_CORES)), trace=trace, **kw
    )
    out = np.empty((B, SQ, D), dtype=np.float32)
    for c in range(N_CORES):
        b, h = divmod(c, 2)
        out[b, h * SQ_SHARD : (h + 1) * SQ_SHARD, :] = _unpermute(
            res.results[c]["out"]
        ).astype(np.float32)
    return out, res


def kernel(**inputs: np.ndarray) -> np.ndarray:
    if os.environ.get("BASS_TRACE"):
        _install_axon_ntff_shim()
    try:
        out, _ = _run(inputs, trace=False)
    except Exception:
        # transient NRT device errors have been observed once across many
        # runs; one retry on a fresh execution
        out, _ = _run(inputs, trace=False)
    return out


if __name__ == "__main__":
    rng = np.random.default_rng(0)
    ins = {
        "context": rng.standard_normal((B, SKV, D)).astype(np.float32),
        "X": rng.standard_normal((B, SQ, D)).astype(np.float32),
        "Wq": (rng.standard_normal((D, D)) / np.sqrt(D)).astype(np.float32),
        "Wk": (rng.standard_normal((D, D)) / np.sqrt(D)).astype(np.float32),
        "Wv": (rng.standard_normal((D, D)) / np.sqrt(D)).astype(np.float32),
    }
    got = kernel(**ins)
    q = ins["X"] @ ins["Wq"].T
    k = ins["context"] @ ins["Wk"].T
    v = ins["context"] @ ins["Wv"].T
    w = np.einsum("bse,bte->bst", q, k) * SCALE
    want = np.einsum("bst,bte->bse", w, v)
    rel = np.linalg.norm(got - want) / np.linalg.norm(want)
    print("rel err vs numpy:", rel)
